# revision 34
# baseline (speedup 1.0000x reference)
"""3-layer GraphSAGE (mean aggr) on Trainium2, 8-core SPMD, fp16 compute.

Strategy (graph/data parallel; src-sharded aggregation + ReduceScatter):
  - Nodes padded 10000 -> 10240, assigned to 80 blocks of 128 by a
    degree-balancing permutation. Core r owns blocks [10r, 10r+10);
    one SPMD program runs on all 8 cores.
  - Layer 1 is dst-sharded (x is replicated input, no collective):
    per-block edge gathers (GPSIMD dma_gather) + one-hot matmuls
    accumulate mean1^T in PSUM; h1 kept node-major in DRAM (gather
    source for L2) and transposed in SBUF (L2 root term lhsT).
  - Layers 2/3 are src-sharded: each core gathers rows of its OWN
    h1/t3 for its out-edges (sorted by dst block; per-block slot
    counts are max across cores so the schedule is SPMD-static, and
    chunks spanning block boundaries get one matmul per block they
    touch), aggregates invdeg-weighted partial sums for ALL 10240
    dst nodes via one-hot matmuls, and a ReduceScatter(add) returns
    each core its own rows summed. RS input bands are stored
    feature-major (the reduction is elementwise, so each 1280-row
    band can hold transposed data): the RS output IS mean^T, fed
    straight to the next matmul as lhsT -- no transposes anywhere.
  - L3 "transform first": t3 = h2@w3l and r3 = h2@w3r + b3 computed
    in the L2 block loop; only t3 [1280,64] rows are gathered and
    partial-aggregated node-major, RS2 output + r3 -> log_softmax.
  - Collectives cost (cost model: 15us + out_bytes/40GBps) drops from
    two AllGathers (146+80.5us) to two ReduceScatters (31.4+19.1us).
  - Activation funcs restricted to {relu, copy, exp, ln} with exp/ln
    batched into separate passes so act-table reloads are minimal.
    PSUM->SBUF partial copies run on the otherwise-idle ACT engine.
  - f32 PSUM accumulation everywhere; fp16 on all wires. End-to-end
    vs the f32 reference: rel err ~3e-4.
"""

import numpy as np

BF = np.float16

N_NODES = 10000
NPAD = 10240
NCORES = 8
P = 128
NB = 10                      # dst blocks per core
PER_CORE = NB * P            # 1280
NBLK = NPAD // P             # 80
D_IN, D_H1, D_H2, D_OUT = 128, 256, 1024, 64

_CACHE = {}
LAST_RESULTS = None          # test harness reads exec_time_ns from here


def _mk_schedule(S):
    """Static L2/L3 aggregation schedule from per-block slot counts S[g].

    Returns (NCH, instances, chunk_insts):
      instances: list of (chunk, g, first, last) in emission order;
      chunk_insts[c]: instance indices for chunk c.
    """
    offs = np.concatenate([[0], np.cumsum(S)])
    T = int(offs[-1])
    NCH = (T + P - 1) // P
    instances = []
    chunk_insts = [[] for _ in range(NCH)]
    for g in range(NBLK):
        lo, hi = int(offs[g]), int(offs[g + 1])
        if hi == lo:
            continue
        c_lo, c_hi = lo // P, (hi - 1) // P
        for c in range(c_lo, c_hi + 1):
            k = len(instances)
            instances.append((c, g, c == c_lo, c == c_hi))
            chunk_insts[c].append(k)
    return NCH, instances, chunk_insts


def _build(MC, S):
    import os
    import concourse.bacc as bacc
    import concourse.mybir as mybir
    import concourse.tile as tile

    abl = set(os.environ.get("KABL", "").split(","))

    f32 = mybir.dt.float32
    bf16 = mybir.dt.float16
    f8 = mybir.dt.float8e4
    i16 = mybir.dt.int16
    nc = bacc.Bacc("TRN2", target_bir_lowering=False, debug=False,
                   num_devices=NCORES)

    NCH, INSTS, CHUNK_INSTS = _mk_schedule(S)
    NINST = len(INSTS)

    xbf = nc.dram_tensor("xbf", [NPAD, D_IN], bf16, kind="ExternalInput")
    xownT = nc.dram_tensor("xownT", [P, PER_CORE], bf16, kind="ExternalInput")
    w1l = nc.dram_tensor("w1l", [D_IN, D_H1], bf16, kind="ExternalInput")
    w1r = nc.dram_tensor("w1r", [D_IN, D_H1], bf16, kind="ExternalInput")
    b1 = nc.dram_tensor("b1", [1, D_H1], bf16, kind="ExternalInput")
    b1t = nc.dram_tensor("b1t", [P, 2], f32, kind="ExternalInput")
    w2l = nc.dram_tensor("w2l", [D_H1, D_H2], bf16, kind="ExternalInput")
    w2r = nc.dram_tensor("w2r", [D_H1, D_H2], bf16, kind="ExternalInput")
    b2t = nc.dram_tensor("b2t", [P, 8], f32, kind="ExternalInput")
    w3lr = nc.dram_tensor("w3lr", [D_H2, P], bf16, kind="ExternalInput")
    b3pad = nc.dram_tensor("b3pad", [1, P], bf16, kind="ExternalInput")
    gidx1 = nc.dram_tensor("gidx1", [P, NB * MC * 8], i16, kind="ExternalInput")
    dl1 = nc.dram_tensor("dl1", [P, NB * MC], f32, kind="ExternalInput")
    iv1 = nc.dram_tensor("iv1", [P, NB * MC], f32, kind="ExternalInput")
    iota_in = nc.dram_tensor("iota_in", [P, P], bf16, kind="ExternalInput")
    gidx2 = nc.dram_tensor("gidx2", [P, NCH * 8], i16, kind="ExternalInput")
    oh8 = nc.dram_tensor("oh8", [P, NINST * P], f8, kind="ExternalInput")
    ivown = nc.dram_tensor("ivown", [P, NB], f32, kind="ExternalInput")
    dvdiag = nc.dram_tensor("dvdiag", [P, NB * P], bf16, kind="ExternalInput")
    outp = nc.dram_tensor("out", [PER_CORE, D_OUT], f32, kind="ExternalOutput")

    EXP = mybir.ActivationFunctionType.Exp
    LN = mybir.ActivationFunctionType.Ln
    RELU = mybir.ActivationFunctionType.Relu
    COPY = mybir.ActivationFunctionType.Copy
    EQ = mybir.AluOpType.is_equal
    MUL = mybir.AluOpType.mult
    SUB = mybir.AluOpType.subtract
    ADD = mybir.AluOpType.add
    MAX = mybir.AluOpType.max
    AXX = mybir.AxisListType.X

    with tile.TileContext(nc) as tc:
        with (
            tc.tile_pool(name="const", bufs=1) as cp,
            tc.tile_pool(name="gath", bufs=3) as gp,
            tc.tile_pool(name="ht", bufs=2) as htp,
            tc.tile_pool(name="meant", bufs=3) as mtp,
            tc.tile_pool(name="hout", bufs=2) as hop,
            tc.tile_pool(name="band2", bufs=2) as b2p,
            tc.tile_pool(name="band3", bufs=2) as b3p,
            tc.tile_pool(name="small", bufs=6) as smp,
            tc.tile_pool(name="soft", bufs=24) as sfp,
            tc.tile_pool(name="psA", bufs=4, space="PSUM") as psA,
            tc.tile_pool(name="psO", bufs=2, space="PSUM") as psO,
            tc.tile_pool(name="psT", bufs=2, space="PSUM") as psT,
            tc.tile_pool(name="dram", bufs=1, space="DRAM") as dram,
        ):
            # ---- constants (gather indices first: L1 gathers wait on them)
            gidx1_sb = cp.tile([P, NB * MC * 8], i16, tag="gidx1")
            nc.sync.dma_start(gidx1_sb[:], gidx1[:])
            iota_t = cp.tile([P, P], bf16, tag="iota")
            nc.sync.dma_start(iota_t[:], iota_in[:])
            ones_t = cp.tile([1, P], bf16, tag="ones")
            nc.vector.memset(ones_t[:], 1.0)
            dum = cp.tile([1, 2], f32, tag="dum")
            nc.vector.memset(dum[:], 1.0)
            nc.scalar.activation(dum[:, 0:1], dum[:, 1:2], LN)
            nc.scalar.activation(dum[:, 0:1], dum[:, 1:2], EXP)
            dl1_sb = cp.tile([P, NB * MC], f32, tag="dl1")
            nc.sync.dma_start(dl1_sb[:], dl1[:])
            iv1_sb = cp.tile([P, NB * MC], f32, tag="iv1")
            nc.sync.dma_start(iv1_sb[:], iv1[:])
            ohc1 = cp.tile([P, NB * MC, P], bf16, tag="ohc1")
            gidx2_sb = cp.tile([P, NCH * 8], i16, tag="gidx2")
            nc.sync.dma_start(gidx2_sb[:], gidx2[:])


            w1l_sb = cp.tile([P, D_H1], bf16, tag="w1l")
            nc.sync.dma_start(w1l_sb[:], w1l[:])
            w1r_sb = cp.tile([P, D_H1], bf16, tag="w1r")
            nc.sync.dma_start(w1r_sb[:], w1r[:])
            b1_sb = cp.tile([1, D_H1], bf16, tag="b1")
            nc.sync.dma_start(b1_sb[:], b1[:])
            b1t_sb = cp.tile([P, 2], f32, tag="b1t")
            nc.sync.dma_start(b1t_sb[:], b1t[:])

            # resident cross-phase SBUF state
            xT_res = cp.tile([P, PER_CORE], bf16, tag="xT")
            nc.sync.dma_start(xT_res[:], xownT[:])
            h1T_res = cp.tile([P, 2, PER_CORE], bf16, tag="h1T")
            r3_res = cp.tile([P, NB, D_OUT], f32, tag="r3")
            ohc8 = cp.tile([P, NINST, P], f8, tag="ohc8")
            ivown_sb = cp.tile([P, NB], f32, tag="ivown")
            nc.sync.dma_start(ivown_sb[:], ivown[:])
            dv_sb = cp.tile([P, NB, P], bf16, tag="dvdiag")
            nc.sync.dma_start(dv_sb[:], dvdiag[:])

            # ---- DRAM intermediates ----
            h1_own = dram.tile([PER_CORE, D_H1], f8, tag="h1o")
            t3_own = dram.tile([PER_CORE, 2 * P], f8, tag="t3o")
            p2d = dram.tile([NCORES, P, NB, D_H1], bf16, tag="p2")
            m1n = dram.tile([P, NB, D_H1], bf16, tag="m1n")
            p3d = dram.tile([NCORES, P, NB, D_OUT], bf16, tag="p3")
            m3d = dram.tile([P, NB, D_OUT], bf16, tag="m3")

            # ================= Layer 1 (dst-sharded, local) =================
            for b in range(NB if "l1" not in abl else 0):
                gath = gp.tile([P, MC, D_IN // 2], f32, tag="gath")
                g0 = 0
                while g0 < MC:
                    gsz = min(8, MC - g0)
                    c0 = (b * MC + g0) * 8
                    nc.gpsimd.dma_gather(
                        gath[:, g0:g0 + gsz, :], xbf[:].bitcast(f32),
                        gidx1_sb[:, c0:c0 + gsz * 8],
                        gsz * P, gsz * P, D_IN // 2, single_packet=False)
                    g0 += gsz
                gathh = gath.bitcast(bf16)
                for c in range(MC):
                    col = b * MC + c
                    nc.vector.tensor_scalar(
                        ohc1[:, col, :], iota_t[:],
                        dl1_sb[:, col:col + 1], iv1_sb[:, col:col + 1],
                        EQ, MUL)
                agg = psA.tile([P, 2, P], f32, tag="agg")
                for c in range(MC):
                    nc.tensor.matmul(agg[:, 0, :], gathh[:, c, :],
                                     ohc1[:, b * MC + c, :],
                                     start=(c == 0), stop=(c == MC - 1))
                meanT = mtp.tile([P, P], bf16, tag="meanT")
                nc.vector.tensor_copy(meanT[:], agg[:, 0, :])

                xT = xT_res[:, b * P:(b + 1) * P]

                # node-major h1 (gather source for L2)
                op = psO.tile([P, D_H1], f32, tag="outp")
                nc.tensor.matmul(op[:], meanT[:], w1l_sb[:],
                                 start=True, stop=False)
                nc.tensor.matmul(op[:], xT, w1r_sb[:],
                                 start=False, stop=False)
                nc.tensor.matmul(op[:], ones_t[:], b1_sb[:],
                                 start=False, stop=True)
                h1blk = hop.tile([P, D_H1], f8, tag="hout")
                nc.scalar.activation(h1blk[:], op[:], RELU)
                nc.sync.dma_start(h1_own[b * P:(b + 1) * P, :], h1blk[:])
                # transposed h1 (L2 root term), SBUF-resident
                for s in range(2):
                    tph = psT.tile([P, P], f32, tag="tp")
                    nc.tensor.matmul(tph[:], w1l_sb[:, s * P:(s + 1) * P],
                                     meanT[:], start=True, stop=False)
                    nc.tensor.matmul(tph[:], w1r_sb[:, s * P:(s + 1) * P],
                                     xT, start=False, stop=True)
                    nc.scalar.activation(h1T_res[:, s, b * P:(b + 1) * P],
                                         tph[:], RELU,
                                         bias=b1t_sb[:, s:s + 1])

            # fp8 0/1 one-hots for L2/L3 from host, landed in queue idle
            # windows: manual waits keep them off L1-critical queue time
            _q = (NINST + 3) // 4
            _eng = [nc.gpsimd, nc.gpsimd, nc.sync, nc.sync]
            _wts = [0.006, 0.013, 0.021, 0.027]
            for _i in range(4):
                _lo = _i * _q
                _hi = min(NINST, _lo + _q)
                if _lo < _hi:
                    with tc.tile_wait_until(_wts[_i]):
                        _eng[_i].dma_start(ohc8[:, _lo:_hi, :],
                                           oh8[:, _lo * P:_hi * P])

            # L2/L3 weights: first needed after L1
            w2l_sb = cp.tile([P, 2, D_H2], bf16, tag="w2l")
            nc.sync.dma_start(w2l_sb[:], w2l.rearrange("(s p) n -> p s n", p=P))
            w2r_sb = cp.tile([P, 2, D_H2], bf16, tag="w2r")
            nc.sync.dma_start(w2r_sb[:], w2r.rearrange("(s p) n -> p s n", p=P))
            b2t_sb = cp.tile([P, 8], f32, tag="b2t")
            nc.sync.dma_start(b2t_sb[:], b2t[:])
            w3lr_sb = cp.tile([P, 8, P], bf16, tag="w3lr")
            nc.sync.dma_start(w3lr_sb[:], w3lr.rearrange("(s p) n -> p s n", p=P))
            b3_sb = cp.tile([1, P], bf16, tag="b3")
            nc.sync.dma_start(b3_sb[:], b3pad[:])

            # ====== Layer 2 partial aggregation (src-sharded) ======
            if "l2a" not in abl:
                # per-block instance lists + the window holding each chunk
                binsts = [[] for _ in range(NBLK)]
                for k, (c, g, _, _) in enumerate(INSTS):
                    binsts[g].append((k, c))
                last_win = [max(c for _, c in bi) // 8 for bi in binsts]
                wtiles = {}
                band_t = None
                w_done = 0
                NW = (NCH + 7) // 8
                DR = mybir.MatmulPerfMode.DoubleRow
                ps = None
                for g in range(NBLK):
                    while w_done <= last_win[g] and w_done < NW:
                        c0 = w_done * 8
                        sz = min(8, NCH - c0)
                        gt = gp.tile([P, 8, D_H1 // 4], f32, name="gath",
                                     tag="gath")
                        nc.gpsimd.dma_gather(
                            gt[:, 0:sz, :], h1_own[:].bitcast(f32),
                            gidx2_sb[:, c0 * 8:(c0 + sz) * 8],
                            sz * P, sz * P, D_H1 // 4, single_packet=False)
                        wtiles[w_done] = gt.bitcast(f8)
                        w_done += 1
                    # pair consecutive chunks (same window) for DoubleRow
                    bi = binsts[g]
                    units = []
                    j = 0
                    while j < len(bi):
                        k, c = bi[j]
                        if (j + 1 < len(bi) and bi[j + 1][1] == c + 1
                                and (c + 1) // 8 == c // 8):
                            units.append((k, c, True))
                            j += 2
                        else:
                            units.append((k, c, False))
                            j += 1
                    half = g % 2
                    if half == 0:
                        ps = psA.tile([P, 2, 2, P], f32, name="agg",
                                      tag="agg")
                    for j, (k, c, dr) in enumerate(units):
                        w, i = c // 8, c % 8
                        if dr:
                            nc.tensor.matmul(
                                ps[:, half], ohc8[:, k:k + 2, :],
                                wtiles[w][:, i:i + 2, :],
                                start=(j == 0), stop=(j == len(units) - 1),
                                perf_mode=DR)
                        else:
                            nc.tensor.matmul(
                                ps[:, half], ohc8[:, k, :],
                                wtiles[w][:, i, :],
                                start=(j == 0), stop=(j == len(units) - 1))
                    r_band, gl = divmod(g, NB)
                    if gl == 0:
                        band_t = b2p.tile([P, NB, D_H1], bf16, tag="band2")
                    if half == 1:
                        # one copy for the block pair
                        if (gl // 2) % 2 == 0:
                            nc.scalar.activation(
                                band_t[:, gl - 1:gl + 1, :], ps[:], COPY)
                        else:
                            nc.vector.tensor_copy(
                                band_t[:, gl - 1:gl + 1, :], ps[:])
                    if gl == NB - 1:
                        nc.sync.dma_start(p2d[r_band], band_t[:])

            if "rs1" not in abl:
                nc.gpsimd.collective_compute(
                    "ReduceScatter", mybir.AluOpType.add,
                    replica_groups=[list(range(NCORES))],
                    ins=[p2d.opt()], outs=[m1n.opt()])

            # PE p-state warm-up: dummy matmuls abutting the end of RS1 so
            # the L2 transform starts at full PE clock (results unused)
            warm = cp.tile([P, P], f32, tag="warm")
            wps = psT.tile([P, 512], f32, name="tp", tag="tp")
            with tc.tile_wait_until(0.080):
                for _i in range(40):
                    nc.tensor.matmul(wps[:, 0:P], iota_t[:], iota_t[:],
                                     start=(_i == 0), stop=(_i == 39))
            nc.vector.tensor_copy(warm[:], wps[:, 0:P])

            # ====== Layer 2 transform + L3 transform (local) ======
            m1n_sb = cp.tile([P, NB, D_H1], bf16, tag="m1ns")
            nc.sync.dma_start(m1n_sb[:, 0:NB // 2, :], m1n[:, 0:NB // 2, :])
            nc.sync.dma_start(m1n_sb[:, NB // 2:, :], m1n[:, NB // 2:, :])
            meanT_sb = cp.tile([P, 2, PER_CORE], bf16, tag="m1Ts")
            zero_sb = cp.tile([P, 1], f32, tag="zero")
            nc.vector.memset(zero_sb[:], 0.0)
            for b in range(NB):
                # transpose + invdeg scale in one matmul: rhs is the
                # per-block diag(invdeg)
                for sl in range(2):
                    pst = psT.tile([P, 512], f32, name="tp", tag="tp")
                    nc.tensor.matmul(pst[:, 0:P],
                                     m1n_sb[:, b, sl * P:(sl + 1) * P],
                                     dv_sb[:, b, :], start=True, stop=True)
                    if sl == 0:
                        nc.scalar.activation(
                            meanT_sb[:, sl, b * P:(b + 1) * P],
                            pst[:, 0:P], COPY)
                    else:
                        nc.vector.tensor_copy(
                            meanT_sb[:, sl, b * P:(b + 1) * P], pst[:, 0:P])

            def emit_t3(b, hT, col0):
                # [t3 | r3] = h2 @ [w3l | w3r] + [0 | b3] (lhsT = h2^T)
                tr = psO.tile([P, D_H1], f32, name="tr", tag="outp")
                for s in range(8):
                    nc.tensor.matmul(tr[:, 0:P], hT[:, s, col0:col0 + P],
                                     w3lr_sb[:, s, :],
                                     start=(s == 0), stop=False)
                nc.tensor.matmul(tr[:, 0:P], ones_t[:], b3_sb[:],
                                 start=False, stop=True)
                t3blk = smp.tile([P, 2 * P], f8, name="t3blk", tag="t3blk")
                nc.vector.tensor_copy(t3blk[:, 0:D_OUT], tr[:, 0:D_OUT])
                nc.vector.memset(t3blk[:, D_OUT:2 * P], 0.0)
                nc.vector.tensor_copy(r3_res[:, b, :], tr[:, D_OUT:2 * D_OUT])
                nc.sync.dma_start(t3_own[b * P:(b + 1) * P, :], t3blk[:])

            # 512-node column groups: fewer, wider matmuls (one PSUM bank)
            GRP = [(0, 512), (512, 512), (1024, 256)]
            if "l2t" not in abl:
                hTg = []
                for (q0, qw) in GRP:
                    hT = htp.tile([P, 8, 512], bf16, name="hT", tag="hT")
                    hTg.append(hT)
                    for s in range(8):
                        hp = psT.tile([P, 512], f32, name="tp", tag="tp")
                        nc.tensor.matmul(hp[:, 0:qw],
                                         w2l_sb[:, 0, s * P:(s + 1) * P],
                                         meanT_sb[:, 0, q0:q0 + qw],
                                         start=True, stop=False)
                        nc.tensor.matmul(hp[:, 0:qw],
                                         w2l_sb[:, 1, s * P:(s + 1) * P],
                                         meanT_sb[:, 1, q0:q0 + qw],
                                         start=False, stop=False)
                        nc.tensor.matmul(hp[:, 0:qw],
                                         w2r_sb[:, 0, s * P:(s + 1) * P],
                                         h1T_res[:, 0, q0:q0 + qw],
                                         start=False, stop=False)
                        nc.tensor.matmul(hp[:, 0:qw],
                                         w2r_sb[:, 1, s * P:(s + 1) * P],
                                         h1T_res[:, 1, q0:q0 + qw],
                                         start=False, stop=True)
                        if s % 2 == 0:
                            nc.scalar.activation(hT[:, s, 0:qw], hp[:, 0:qw],
                                                 RELU,
                                                 bias=b2t_sb[:, s:s + 1])
                        else:
                            nc.vector.tensor_scalar(hT[:, s, 0:qw],
                                                    hp[:, 0:qw],
                                                    b2t_sb[:, s:s + 1],
                                                    zero_sb[:, 0:1], ADD, MAX)
                    # t3/r3 for the blocks inside this column group
                    for bq in range(qw // P):
                        b = q0 // P + bq
                        emit_t3(b, hT, bq * P)

            # ====== Layer 3 partial aggregation (src-sharded) ======
            if "l3a" not in abl:
                binsts = [[] for _ in range(NBLK)]
                for k, (c, g, _, _) in enumerate(INSTS):
                    binsts[g].append((k, c))
                last_win = [max(c for _, c in bi) // 8 for bi in binsts]
                wtiles = {}
                band_t = None
                w_done = 0
                NW = (NCH + 7) // 8
                for g in range(NBLK):
                    while w_done <= last_win[g] and w_done < NW:
                        c0 = w_done * 8
                        sz = min(8, NCH - c0)
                        gt = gp.tile([P, 8, P // 2], f32, name="gath3",
                                     tag="gath")
                        nc.gpsimd.dma_gather(
                            gt[:, 0:sz, :], t3_own[:].bitcast(f32),
                            gidx2_sb[:, c0 * 8:(c0 + sz) * 8],
                            sz * P, sz * P, P // 2, single_packet=False)
                        wtiles[w_done] = gt.bitcast(f8)
                        w_done += 1
                    ps = psA.tile([P, 2, P], f32, name="agg", tag="agg")
                    n = len(binsts[g])
                    for j, (k, c) in enumerate(binsts[g]):
                        nc.tensor.matmul(ps[:, 0, 0:D_OUT],
                                         ohc8[:, k, :],
                                         wtiles[c // 8][:, c % 8, 0:D_OUT],
                                         start=(j == 0), stop=(j == n - 1))
                    r_band, gl = divmod(g, NB)
                    if gl == 0:
                        band_t = b3p.tile([P, NB, D_OUT], bf16, tag="band3")
                    if gl % 2 == 0:
                        nc.scalar.activation(
                            band_t[:, gl, :], ps[:, 0, 0:D_OUT], COPY)
                    else:
                        nc.vector.tensor_copy(
                            band_t[:, gl, :], ps[:, 0, 0:D_OUT])
                    if gl == NB - 1:
                        nc.sync.dma_start(p3d[r_band], band_t[:])

            if "rs2" not in abl:
                nc.gpsimd.collective_compute(
                    "ReduceScatter", mybir.AluOpType.add,
                    replica_groups=[list(range(NCORES))],
                    ins=[p3d.opt()], outs=[m3d.opt()])

            # ====== final: y = mean3 + r3, log_softmax, store ======
            if "fin" not in abl:
                m3_sb = cp.tile([P, NB, D_OUT], bf16, tag="m3s")
                nc.sync.dma_start(m3_sb[:], m3d[:])
                m3f = cp.tile([P, NB, D_OUT], f32, tag="m3f")
                nc.vector.tensor_copy(m3f[:], m3_sb[:])
                ys, negs = [], []
                ssum_all = cp.tile([P, NB], f32, tag="ssum_all")
                ls_all = cp.tile([P, NB], f32, tag="ls_all")
                for b in range(NB):
                    ym = sfp.tile([P, D_OUT], f32, tag="ym")
                    nc.vector.tensor_scalar_mul(ym[:], m3f[:, b, :],
                                                ivown_sb[:, b:b + 1])
                    y = sfp.tile([P, D_OUT], f32, tag="y")
                    nc.vector.tensor_tensor(y[:], ym[:],
                                            r3_res[:, b, :], ADD)
                    negm = sfp.tile([P, 1], f32, tag="negm")
                    nc.vector.tensor_reduce(negm[:], y[:], AXX, MAX,
                                            negate=True)
                    ys.append(y)
                    negs.append(negm)
                for b in range(NB):        # all Exp together (one act table)
                    e = sfp.tile([P, D_OUT], f32, tag="e")
                    nc.scalar.activation(e[:], ys[b][:], EXP,
                                         bias=negs[b][:, 0:1],
                                         scale=1.0,
                                         accum_out=ssum_all[:, b:b + 1])
                nc.scalar.activation(ls_all[:], ssum_all[:], LN)
                for b in range(NB):
                    ob = sfp.tile([P, D_OUT], f32, name="ob", tag="ob")
                    nc.vector.tensor_scalar(ob[:], ys[b][:],
                                            negs[b][:, 0:1],
                                            ls_all[:, b:b + 1],
                                            ADD, SUB)
                    nc.sync.dma_start(outp[b * P:(b + 1) * P, :], ob[:])

    nc.compile()
    return nc


def _wrap16(a):
    """idx i -> partition i%16, col i//16; replicated to 128 partitions."""
    w = a.reshape(-1, 16).T
    return np.ascontiguousarray(np.tile(w, (8, 1)))


def _balanced_perm(deg):
    """Assign nodes to 80 blocks of 128 so block in-degree sums are even."""
    import heapq
    order = np.argsort(-deg, kind="stable")
    heap = [(0, 0, g) for g in range(NBLK)]
    heapq.heapify(heap)
    newpos = np.empty(NPAD, np.int64)
    fill = np.zeros(NBLK, np.int64)
    for n in order:
        s, _, g = heapq.heappop(heap)
        newpos[n] = g * P + fill[g]
        fill[g] += 1
        if fill[g] < P:
            heapq.heappush(heap, (s + int(deg[n]), int(fill[g]), g))
    return newpos


def _prep(x, edge_index):
    src = np.asarray(edge_index[0], dtype=np.int64)
    dst = np.asarray(edge_index[1], dtype=np.int64)
    deg = np.bincount(dst, minlength=NPAD).astype(np.float64)
    invdeg_n = (1.0 / np.maximum(deg, 1.0)).astype(np.float32)

    newpos = _balanced_perm(deg)
    oldnode = np.empty(NPAD, np.int64)
    oldnode[newpos] = np.arange(NPAD)
    psrc = newpos[src]
    pdst = newpos[dst]

    # ---------- L1: dst-sharded per-block chunks (global src gather) ----
    order = np.argsort(pdst, kind="stable")
    dsts = pdst[order]
    srcs = psrc[order]
    inv_e = invdeg_n[dst[order]]
    starts = np.searchsorted(dsts, np.arange(0, NPAD + P, P))
    cnt = starts[1:] - starts[:-1]
    MC = max(1, int(np.ceil(cnt.max() / P)))

    l1_per_core = []
    for r in range(NCORES):
        gparts, dparts, iparts = [], [], []
        for j in range(NB):
            g = r * NB + j
            lo, hi = starts[g], starts[g + 1]
            n = hi - lo
            o2 = lo + np.argsort(srcs[lo:hi], kind="stable")
            sg = np.zeros(MC * P, dtype=np.int16)
            dg = np.full(MC * P, -1.0, dtype=np.float32)
            ig = np.zeros(MC * P, dtype=np.float32)
            sg[:n] = srcs[o2].astype(np.int16)
            dg[:n] = (dsts[o2] - g * P).astype(np.float32)
            ig[:n] = inv_e[o2]
            gparts.append(_wrap16(sg))
            dparts.append(np.ascontiguousarray(dg.reshape(MC, P).T))
            iparts.append(np.ascontiguousarray(ig.reshape(MC, P).T))
        l1_per_core.append((
            np.concatenate(gparts, axis=1),
            np.concatenate(dparts, axis=1),
            np.concatenate(iparts, axis=1),
        ))

    # ---------- L2/L3: src-sharded shared schedule (local src gather) ---
    src_core = psrc // PER_CORE
    srcloc = psrc % PER_CORE
    g_all = pdst // P
    dloc_all = (pdst % P).astype(np.float32)
    inv_all = invdeg_n[dst]

    E = np.zeros((NCORES, NBLK), dtype=np.int64)
    for r in range(NCORES):
        E[r] = np.bincount(g_all[src_core == r], minlength=NBLK)
    S = np.maximum(1, E.max(axis=0))          # slots per block (static)
    offs = np.concatenate([[0], np.cumsum(S)])
    T = int(offs[-1])
    NCH = (T + P - 1) // P
    Tpad = NCH * P

    NCHsched, INSTS, _ = _mk_schedule(tuple(int(v) for v in S))
    assert NCHsched == NCH
    NINST = len(INSTS)

    l2_per_core = []
    for r in range(NCORES):
        m = src_core == r
        g_r = g_all[m]
        sl_r = srcloc[m]
        dl_r = dloc_all[m]
        iv_r = inv_all[m]
        o2 = np.lexsort((sl_r, g_r))
        g_s, sl_s, dl_s, iv_s = g_r[o2], sl_r[o2], dl_r[o2], iv_r[o2]
        # position within block group
        gstart = np.searchsorted(g_s, np.arange(NBLK))
        within = np.arange(len(g_s)) - gstart[g_s]
        flat_pos = offs[g_s] + within

        sgf = np.zeros(Tpad, dtype=np.int16)
        dlf = np.full(Tpad, -1.0, dtype=np.float32)
        ivf = np.zeros(Tpad, dtype=np.float32)
        sgf[flat_pos] = sl_s.astype(np.int16)
        dlf[flat_pos] = dl_s
        ivf[flat_pos] = iv_s

        # per-instance one-hots [slot, dst-local], invdeg folded in
        dl_i = np.full((NINST, P), -1.0, dtype=np.float32)
        iv_i = np.zeros((NINST, P), dtype=np.float32)
        for k, (c, g, _, _) in enumerate(INSTS):
            s0 = c * P
            sel = np.arange(s0, s0 + P)
            inb = (sel >= offs[g]) & (sel < offs[g + 1])
            dl_i[k, inb] = dlf[sel[inb]]
            iv_i[k, inb] = ivf[sel[inb]]
        import ml_dtypes
        ohm = (dl_i[:, :, None]
               == np.arange(P, dtype=np.float32)[None, None, :])
        # 0/1 one-hots [NINST, slot, dst] -> SBUF [slot(part), inst, dst]
        oh8 = np.ascontiguousarray(
            ohm.astype(ml_dtypes.float8_e4m3)
            .transpose(1, 0, 2).reshape(P, NINST * P))

        l2_per_core.append((_wrap16(sgf), oh8))

    xp = np.zeros((NPAD, D_IN), dtype=np.float32)
    xp[:N_NODES] = x
    xp = xp[oldnode]           # permuted node order
    ivp = invdeg_n[oldnode]    # invdeg in permuted node order
    return (xp, ivp, l1_per_core, l2_per_core, MC,
            tuple(int(v) for v in S), newpos)


def _make_in_maps(x, edge_index, w1l, w1r, b1, w2l, w2r, b2, w3l, w3r, b3):
    x = np.ascontiguousarray(np.asarray(x, dtype=np.float32))
    xp, ivp, l1pc, l2pc, MC, S, newpos = _prep(x, np.asarray(edge_index))

    iota = np.tile(np.arange(P, dtype=BF), (P, 1))
    b1v = np.asarray(b1, np.float32).reshape(-1)
    b2v = np.asarray(b2, np.float32).reshape(-1)
    xbf = xp.astype(BF)
    common = {
        "xbf": xbf,
        "w1l": np.asarray(w1l, np.float32).astype(BF),
        "w1r": np.asarray(w1r, np.float32).astype(BF),
        "b1": b1v.reshape(1, D_H1).astype(BF),
        "b1t": np.ascontiguousarray(b1v.reshape(2, P).T),
        "w2l": np.asarray(w2l, np.float32).astype(BF),
        "w2r": np.asarray(w2r, np.float32).astype(BF),
        "b2t": np.ascontiguousarray(b2v.reshape(8, P).T),
        "w3lr": np.ascontiguousarray(np.concatenate(
            [np.asarray(w3l, np.float32), np.asarray(w3r, np.float32)],
            axis=1)).astype(BF),
        "b3pad": np.concatenate(
            [np.zeros(D_OUT, np.float32),
             np.asarray(b3, np.float32).reshape(-1)]).reshape(1, P).astype(BF),
        "iota_in": np.ascontiguousarray(iota),
    }
    in_maps = []
    for r in range(NCORES):
        g1, d1, i1 = l1pc[r]
        g2, o8 = l2pc[r]
        mdict = dict(common)
        mdict["xownT"] = np.ascontiguousarray(
            xbf[r * PER_CORE:(r + 1) * PER_CORE].T)
        mdict["gidx1"] = g1
        mdict["dl1"] = d1
        mdict["iv1"] = i1
        mdict["gidx2"] = g2
        mdict["oh8"] = o8
        ivr = ivp[r * PER_CORE:(r + 1) * PER_CORE].reshape(NB, P)
        mdict["ivown"] = np.ascontiguousarray(ivr.T)
        dvd = np.zeros((NB, P, P), dtype=BF)
        for b in range(NB):
            np.fill_diagonal(dvd[b], ivr[b].astype(BF))
        mdict["dvdiag"] = np.ascontiguousarray(
            dvd.transpose(1, 0, 2).reshape(P, NB * P))
        in_maps.append(mdict)
    return in_maps, (MC, S), newpos


def kernel(x, edge_index, w1l, w1r, b1, w2l, w2r, b2, w3l, w3r, b3):
    global LAST_RESULTS
    import os
    from concourse.bass_utils import run_bass_kernel_spmd

    if os.environ.get("BASS_TRACE"):
        try:
            import antenv.axon_hooks  # noqa: F401
        except ImportError:
            os.environ.pop("BASS_TRACE", None)  # no NTFF hook here

    in_maps, key, newpos = _make_in_maps(x, edge_index, w1l, w1r, b1, w2l,
                                         w2r, b2, w3l, w3r, b3)
    if key not in _CACHE:
        _CACHE[key] = _build(key[0], key[1])
    nc = _CACHE[key]

    res = run_bass_kernel_spmd(nc, in_maps, core_ids=list(range(NCORES)))
    LAST_RESULTS = res
    out = np.concatenate([res.results[r]["out"] for r in range(NCORES)], axis=0)
    return np.ascontiguousarray(out[newpos[:N_NODES]])


# revision 35
# speedup vs baseline: 1.0210x; 1.0210x over previous
"""3-layer GraphSAGE (mean aggr) on Trainium2, 8-core SPMD, fp16 compute.

Strategy (graph/data parallel; src-sharded aggregation + ReduceScatter):
  - Nodes padded 10000 -> 10240, assigned to 80 blocks of 128 by a
    degree-balancing permutation. Core r owns blocks [10r, 10r+10);
    one SPMD program runs on all 8 cores.
  - Layer 1 is dst-sharded (x is replicated input, no collective):
    per-block edge gathers (GPSIMD dma_gather) + one-hot matmuls
    accumulate mean1^T in PSUM; h1 kept node-major in DRAM (gather
    source for L2) and transposed in SBUF (L2 root term lhsT).
  - Layers 2/3 are src-sharded: each core gathers rows of its OWN
    h1/t3 for its out-edges (sorted by dst block; per-block slot
    counts are max across cores so the schedule is SPMD-static, and
    chunks spanning block boundaries get one matmul per block they
    touch), aggregates invdeg-weighted partial sums for ALL 10240
    dst nodes via one-hot matmuls, and a ReduceScatter(add) returns
    each core its own rows summed. RS input bands are stored
    feature-major (the reduction is elementwise, so each 1280-row
    band can hold transposed data): the RS output IS mean^T, fed
    straight to the next matmul as lhsT -- no transposes anywhere.
  - L3 "transform first": t3 = h2@w3l and r3 = h2@w3r + b3 computed
    in the L2 block loop; only t3 [1280,64] rows are gathered and
    partial-aggregated node-major, RS2 output + r3 -> log_softmax.
  - Collectives cost (cost model: 15us + out_bytes/40GBps) drops from
    two AllGathers (146+80.5us) to two ReduceScatters (31.4+19.1us).
  - Activation funcs restricted to {relu, copy, exp, ln} with exp/ln
    batched into separate passes so act-table reloads are minimal.
    PSUM->SBUF partial copies run on the otherwise-idle ACT engine.
  - f32 PSUM accumulation everywhere; fp16 on all wires. End-to-end
    vs the f32 reference: rel err ~3e-4.
"""

import numpy as np

BF = np.float16

N_NODES = 10000
NPAD = 10240
NCORES = 8
P = 128
NB = 10                      # dst blocks per core
PER_CORE = NB * P            # 1280
NBLK = NPAD // P             # 80
D_IN, D_H1, D_H2, D_OUT = 128, 256, 1024, 64

_CACHE = {}
LAST_RESULTS = None          # test harness reads exec_time_ns from here


def _mk_schedule(S):
    """Static L2/L3 aggregation schedule from per-block slot counts S[g].

    Returns (NCH, instances, chunk_insts):
      instances: list of (chunk, g, first, last) in emission order;
      chunk_insts[c]: instance indices for chunk c.
    """
    offs = np.concatenate([[0], np.cumsum(S)])
    T = int(offs[-1])
    NCH = (T + P - 1) // P
    instances = []
    chunk_insts = [[] for _ in range(NCH)]
    for g in range(NBLK):
        lo, hi = int(offs[g]), int(offs[g + 1])
        if hi == lo:
            continue
        c_lo, c_hi = lo // P, (hi - 1) // P
        for c in range(c_lo, c_hi + 1):
            k = len(instances)
            instances.append((c, g, c == c_lo, c == c_hi))
            chunk_insts[c].append(k)
    return NCH, instances, chunk_insts


def _build(MC, S):
    import os
    import concourse.bacc as bacc
    import concourse.mybir as mybir
    import concourse.tile as tile

    abl = set(os.environ.get("KABL", "").split(","))

    f32 = mybir.dt.float32
    bf16 = mybir.dt.float16
    f8 = mybir.dt.float8e4
    i16 = mybir.dt.int16
    nc = bacc.Bacc("TRN2", target_bir_lowering=False, debug=False,
                   num_devices=NCORES)

    NCH, INSTS, CHUNK_INSTS = _mk_schedule(S)
    NINST = len(INSTS)

    xbf = nc.dram_tensor("xbf", [NPAD, D_IN], bf16, kind="ExternalInput")
    xownT = nc.dram_tensor("xownT", [P, PER_CORE], bf16, kind="ExternalInput")
    w1l = nc.dram_tensor("w1l", [D_IN, D_H1], bf16, kind="ExternalInput")
    w1r = nc.dram_tensor("w1r", [D_IN, D_H1], bf16, kind="ExternalInput")
    b1 = nc.dram_tensor("b1", [1, D_H1], bf16, kind="ExternalInput")
    b1t = nc.dram_tensor("b1t", [P, 2], f32, kind="ExternalInput")
    w2l = nc.dram_tensor("w2l", [D_H1, D_H2], bf16, kind="ExternalInput")
    w2r = nc.dram_tensor("w2r", [D_H1, D_H2], bf16, kind="ExternalInput")
    b2t = nc.dram_tensor("b2t", [P, 8], f32, kind="ExternalInput")
    w3lr = nc.dram_tensor("w3lr", [D_H2, P], bf16, kind="ExternalInput")
    b3pad = nc.dram_tensor("b3pad", [1, P], bf16, kind="ExternalInput")
    gidx1 = nc.dram_tensor("gidx1", [P, NB * MC * 8], i16, kind="ExternalInput")
    dl1 = nc.dram_tensor("dl1", [P, NB * MC], f32, kind="ExternalInput")
    iv1 = nc.dram_tensor("iv1", [P, NB * MC], f32, kind="ExternalInput")
    iota_in = nc.dram_tensor("iota_in", [P, P], bf16, kind="ExternalInput")
    gidx2 = nc.dram_tensor("gidx2", [P, NCH * 8], i16, kind="ExternalInput")
    oh8 = nc.dram_tensor("oh8", [P, NINST * P], f8, kind="ExternalInput")
    ivown = nc.dram_tensor("ivown", [P, NB], f32, kind="ExternalInput")
    dvdiag = nc.dram_tensor("dvdiag", [P, NB * P], bf16, kind="ExternalInput")
    outp = nc.dram_tensor("out", [P, NB * D_OUT], f32,
                          kind="ExternalOutput")

    EXP = mybir.ActivationFunctionType.Exp
    LN = mybir.ActivationFunctionType.Ln
    RELU = mybir.ActivationFunctionType.Relu
    COPY = mybir.ActivationFunctionType.Copy
    EQ = mybir.AluOpType.is_equal
    MUL = mybir.AluOpType.mult
    SUB = mybir.AluOpType.subtract
    ADD = mybir.AluOpType.add
    MAX = mybir.AluOpType.max
    AXX = mybir.AxisListType.X

    with tile.TileContext(nc) as tc:
        with (
            tc.tile_pool(name="const", bufs=1) as cp,
            tc.tile_pool(name="gath", bufs=3) as gp,
            tc.tile_pool(name="ht", bufs=2) as htp,
            tc.tile_pool(name="meant", bufs=3) as mtp,
            tc.tile_pool(name="hout", bufs=2) as hop,
            tc.tile_pool(name="band2", bufs=2) as b2p,
            tc.tile_pool(name="band3", bufs=2) as b3p,
            tc.tile_pool(name="small", bufs=6) as smp,
            tc.tile_pool(name="soft", bufs=24) as sfp,
            tc.tile_pool(name="psA", bufs=4, space="PSUM") as psA,
            tc.tile_pool(name="psO", bufs=2, space="PSUM") as psO,
            tc.tile_pool(name="psT", bufs=2, space="PSUM") as psT,
            tc.tile_pool(name="dram", bufs=1, space="DRAM") as dram,
        ):
            # ---- constants (gather indices first: L1 gathers wait on them)
            gidx1_sb = cp.tile([P, NB * MC * 8], i16, tag="gidx1")
            nc.sync.dma_start(gidx1_sb[:], gidx1[:])
            iota_t = cp.tile([P, P], bf16, tag="iota")
            nc.sync.dma_start(iota_t[:], iota_in[:])
            ones_t = cp.tile([1, P], bf16, tag="ones")
            nc.vector.memset(ones_t[:], 1.0)
            dum = cp.tile([1, 2], f32, tag="dum")
            nc.vector.memset(dum[:], 1.0)
            nc.scalar.activation(dum[:, 0:1], dum[:, 1:2], LN)
            nc.scalar.activation(dum[:, 0:1], dum[:, 1:2], EXP)
            dl1_sb = cp.tile([P, NB * MC], f32, tag="dl1")
            nc.sync.dma_start(dl1_sb[:], dl1[:])
            iv1_sb = cp.tile([P, NB * MC], f32, tag="iv1")
            nc.sync.dma_start(iv1_sb[:], iv1[:])
            ohc1 = cp.tile([P, NB * MC, P], bf16, tag="ohc1")
            gidx2_sb = cp.tile([P, NCH * 8], i16, tag="gidx2")
            nc.sync.dma_start(gidx2_sb[:], gidx2[:])


            w1l_sb = cp.tile([P, D_H1], bf16, tag="w1l")
            nc.sync.dma_start(w1l_sb[:], w1l[:])
            w1r_sb = cp.tile([P, D_H1], bf16, tag="w1r")
            nc.sync.dma_start(w1r_sb[:], w1r[:])
            b1_sb = cp.tile([1, D_H1], bf16, tag="b1")
            nc.sync.dma_start(b1_sb[:], b1[:])
            b1t_sb = cp.tile([P, 2], f32, tag="b1t")
            nc.sync.dma_start(b1t_sb[:], b1t[:])

            # resident cross-phase SBUF state
            xT_res = cp.tile([P, PER_CORE], bf16, tag="xT")
            nc.sync.dma_start(xT_res[:], xownT[:])
            h1T_res = cp.tile([P, 2, PER_CORE], bf16, tag="h1T")
            r3_res = cp.tile([P, NB, D_OUT], f32, tag="r3")
            ohc8 = cp.tile([P, NINST, P], f8, tag="ohc8")
            ivown_sb = cp.tile([P, NB], f32, tag="ivown")
            nc.sync.dma_start(ivown_sb[:], ivown[:])
            dv_sb = cp.tile([P, NB, P], bf16, tag="dvdiag")
            nc.sync.dma_start(dv_sb[:], dvdiag[:])

            # ---- DRAM intermediates ----
            h1_own = dram.tile([PER_CORE, D_H1], f8, tag="h1o")
            t3_own = dram.tile([PER_CORE, 2 * P], f8, tag="t3o")
            p2d = dram.tile([NCORES, P, NB, D_H1], bf16, tag="p2")
            m1n = dram.tile([P, NB, D_H1], bf16, tag="m1n")
            p3d = dram.tile([NCORES, P, NB, D_OUT], bf16, tag="p3")
            m3d = dram.tile([P, NB, D_OUT], bf16, tag="m3")

            # ================= Layer 1 (dst-sharded, local) =================
            for b in range(NB if "l1" not in abl else 0):
                gath = gp.tile([P, MC, D_IN // 2], f32, tag="gath")
                g0 = 0
                while g0 < MC:
                    gsz = min(8, MC - g0)
                    c0 = (b * MC + g0) * 8
                    nc.gpsimd.dma_gather(
                        gath[:, g0:g0 + gsz, :], xbf[:].bitcast(f32),
                        gidx1_sb[:, c0:c0 + gsz * 8],
                        gsz * P, gsz * P, D_IN // 2, single_packet=False)
                    g0 += gsz
                gathh = gath.bitcast(bf16)
                for c in range(MC):
                    col = b * MC + c
                    nc.vector.tensor_scalar(
                        ohc1[:, col, :], iota_t[:],
                        dl1_sb[:, col:col + 1], iv1_sb[:, col:col + 1],
                        EQ, MUL)
                agg = psA.tile([P, 2, P], f32, tag="agg")
                for c in range(MC):
                    nc.tensor.matmul(agg[:, 0, :], gathh[:, c, :],
                                     ohc1[:, b * MC + c, :],
                                     start=(c == 0), stop=(c == MC - 1))
                meanT = mtp.tile([P, P], bf16, tag="meanT")
                nc.vector.tensor_copy(meanT[:], agg[:, 0, :])

                xT = xT_res[:, b * P:(b + 1) * P]

                # node-major h1 (gather source for L2)
                op = psO.tile([P, D_H1], f32, tag="outp")
                nc.tensor.matmul(op[:], meanT[:], w1l_sb[:],
                                 start=True, stop=False)
                nc.tensor.matmul(op[:], xT, w1r_sb[:],
                                 start=False, stop=False)
                nc.tensor.matmul(op[:], ones_t[:], b1_sb[:],
                                 start=False, stop=True)
                h1blk = hop.tile([P, D_H1], f8, tag="hout")
                nc.scalar.activation(h1blk[:], op[:], RELU)
                nc.sync.dma_start(h1_own[b * P:(b + 1) * P, :], h1blk[:])
                # transposed h1 (L2 root term), SBUF-resident
                for s in range(2):
                    tph = psT.tile([P, P], f32, tag="tp")
                    nc.tensor.matmul(tph[:], w1l_sb[:, s * P:(s + 1) * P],
                                     meanT[:], start=True, stop=False)
                    nc.tensor.matmul(tph[:], w1r_sb[:, s * P:(s + 1) * P],
                                     xT, start=False, stop=True)
                    nc.scalar.activation(h1T_res[:, s, b * P:(b + 1) * P],
                                         tph[:], RELU,
                                         bias=b1t_sb[:, s:s + 1])

            # fp8 0/1 one-hots for L2/L3 from host, landed in queue idle
            # windows: manual waits keep them off L1-critical queue time
            _q = (NINST + 3) // 4
            _eng = [nc.gpsimd, nc.gpsimd, nc.sync, nc.sync]
            _wts = [0.006, 0.013, 0.021, 0.027]
            for _i in range(4):
                _lo = _i * _q
                _hi = min(NINST, _lo + _q)
                if _lo < _hi:
                    with tc.tile_wait_until(_wts[_i]):
                        _eng[_i].dma_start(ohc8[:, _lo:_hi, :],
                                           oh8[:, _lo * P:_hi * P])

            # L2/L3 weights: first needed after L1
            w2l_sb = cp.tile([P, 2, D_H2], bf16, tag="w2l")
            nc.sync.dma_start(w2l_sb[:], w2l.rearrange("(s p) n -> p s n", p=P))
            w2r_sb = cp.tile([P, 2, D_H2], bf16, tag="w2r")
            nc.sync.dma_start(w2r_sb[:], w2r.rearrange("(s p) n -> p s n", p=P))
            b2t_sb = cp.tile([P, 8], f32, tag="b2t")
            nc.sync.dma_start(b2t_sb[:], b2t[:])
            w3lr_sb = cp.tile([P, 8, P], bf16, tag="w3lr")
            nc.sync.dma_start(w3lr_sb[:], w3lr.rearrange("(s p) n -> p s n", p=P))
            b3_sb = cp.tile([1, P], bf16, tag="b3")
            nc.sync.dma_start(b3_sb[:], b3pad[:])

            # ====== Layer 2 partial aggregation (src-sharded) ======
            if "l2a" not in abl:
                # per-block instance lists + the window holding each chunk
                binsts = [[] for _ in range(NBLK)]
                for k, (c, g, _, _) in enumerate(INSTS):
                    binsts[g].append((k, c))
                last_win = [max(c for _, c in bi) // 8 for bi in binsts]
                wtiles = {}
                band_t = None
                w_done = 0
                NW = (NCH + 7) // 8
                DR = mybir.MatmulPerfMode.DoubleRow
                ps = None
                for g in range(NBLK):
                    while w_done <= last_win[g] and w_done < NW:
                        c0 = w_done * 8
                        sz = min(8, NCH - c0)
                        gt = gp.tile([P, 8, D_H1 // 4], f32, name="gath",
                                     tag="gath")
                        nc.gpsimd.dma_gather(
                            gt[:, 0:sz, :], h1_own[:].bitcast(f32),
                            gidx2_sb[:, c0 * 8:(c0 + sz) * 8],
                            sz * P, sz * P, D_H1 // 4, single_packet=False)
                        wtiles[w_done] = gt.bitcast(f8)
                        w_done += 1
                    # pair consecutive chunks (same window) for DoubleRow
                    bi = binsts[g]
                    units = []
                    j = 0
                    while j < len(bi):
                        k, c = bi[j]
                        if (j + 1 < len(bi) and bi[j + 1][1] == c + 1
                                and (c + 1) // 8 == c // 8):
                            units.append((k, c, True))
                            j += 2
                        else:
                            units.append((k, c, False))
                            j += 1
                    half = g % 2
                    if half == 0:
                        ps = psA.tile([P, 2, 2, P], f32, name="agg",
                                      tag="agg")
                    for j, (k, c, dr) in enumerate(units):
                        w, i = c // 8, c % 8
                        if dr:
                            nc.tensor.matmul(
                                ps[:, half], ohc8[:, k:k + 2, :],
                                wtiles[w][:, i:i + 2, :],
                                start=(j == 0), stop=(j == len(units) - 1),
                                perf_mode=DR)
                        else:
                            nc.tensor.matmul(
                                ps[:, half], ohc8[:, k, :],
                                wtiles[w][:, i, :],
                                start=(j == 0), stop=(j == len(units) - 1))
                    r_band, gl = divmod(g, NB)
                    if gl == 0:
                        band_t = b2p.tile([P, NB, D_H1], bf16, tag="band2")
                    if half == 1:
                        # one copy for the block pair
                        if (gl // 2) % 2 == 0:
                            nc.scalar.activation(
                                band_t[:, gl - 1:gl + 1, :], ps[:], COPY)
                        else:
                            nc.vector.tensor_copy(
                                band_t[:, gl - 1:gl + 1, :], ps[:])
                    if gl == NB - 1:
                        nc.sync.dma_start(p2d[r_band], band_t[:])

            if "rs1" not in abl:
                nc.gpsimd.collective_compute(
                    "ReduceScatter", mybir.AluOpType.add,
                    replica_groups=[list(range(NCORES))],
                    ins=[p2d.opt()], outs=[m1n.opt()])

            # PE p-state warm-up: dummy matmuls abutting the end of RS1 so
            # the L2 transform starts at full PE clock (results unused)
            warm = cp.tile([P, P], f32, tag="warm")
            wps = psT.tile([P, 512], f32, name="tp", tag="tp")
            with tc.tile_wait_until(0.080):
                for _i in range(40):
                    nc.tensor.matmul(wps[:, 0:P], iota_t[:], iota_t[:],
                                     start=(_i == 0), stop=(_i == 39))
            nc.vector.tensor_copy(warm[:], wps[:, 0:P])

            # ====== Layer 2 transform + L3 transform (local) ======
            m1n_sb = cp.tile([P, NB, D_H1], bf16, tag="m1ns")
            nc.sync.dma_start(m1n_sb[:, 0:NB // 2, :], m1n[:, 0:NB // 2, :])
            nc.sync.dma_start(m1n_sb[:, NB // 2:, :], m1n[:, NB // 2:, :])
            meanT_sb = cp.tile([P, 2, PER_CORE], bf16, tag="m1Ts")
            zero_sb = cp.tile([P, 1], f32, tag="zero")
            nc.vector.memset(zero_sb[:], 0.0)
            for b in range(NB):
                # transpose + invdeg scale in one matmul: rhs is the
                # per-block diag(invdeg)
                for sl in range(2):
                    pst = psT.tile([P, 512], f32, name="tp", tag="tp")
                    nc.tensor.matmul(pst[:, 0:P],
                                     m1n_sb[:, b, sl * P:(sl + 1) * P],
                                     dv_sb[:, b, :], start=True, stop=True)
                    if sl == 0:
                        nc.scalar.activation(
                            meanT_sb[:, sl, b * P:(b + 1) * P],
                            pst[:, 0:P], COPY)
                    else:
                        nc.vector.tensor_copy(
                            meanT_sb[:, sl, b * P:(b + 1) * P], pst[:, 0:P])

            def emit_t3(b, hT, col0):
                # [t3 | r3] = h2 @ [w3l | w3r] + [0 | b3] (lhsT = h2^T)
                tr = psO.tile([P, D_H1], f32, name="tr", tag="outp")
                for s in range(8):
                    nc.tensor.matmul(tr[:, 0:P], hT[:, s, col0:col0 + P],
                                     w3lr_sb[:, s, :],
                                     start=(s == 0), stop=False)
                nc.tensor.matmul(tr[:, 0:P], ones_t[:], b3_sb[:],
                                 start=False, stop=True)
                t3blk = smp.tile([P, 2 * P], f8, name="t3blk", tag="t3blk")
                nc.vector.tensor_copy(t3blk[:, 0:D_OUT], tr[:, 0:D_OUT])
                nc.vector.memset(t3blk[:, D_OUT:2 * P], 0.0)
                nc.vector.tensor_copy(r3_res[:, b, :], tr[:, D_OUT:2 * D_OUT])
                nc.sync.dma_start(t3_own[b * P:(b + 1) * P, :], t3blk[:])

            # 512-node column groups: fewer, wider matmuls (one PSUM bank)
            GRP = [(0, 512), (512, 512), (1024, 256)]
            if "l2t" not in abl:
                hTg = []
                for (q0, qw) in GRP:
                    hT = htp.tile([P, 8, 512], bf16, name="hT", tag="hT")
                    hTg.append(hT)
                    for s in range(8):
                        hp = psT.tile([P, 512], f32, name="tp", tag="tp")
                        nc.tensor.matmul(hp[:, 0:qw],
                                         w2l_sb[:, 0, s * P:(s + 1) * P],
                                         meanT_sb[:, 0, q0:q0 + qw],
                                         start=True, stop=False)
                        nc.tensor.matmul(hp[:, 0:qw],
                                         w2l_sb[:, 1, s * P:(s + 1) * P],
                                         meanT_sb[:, 1, q0:q0 + qw],
                                         start=False, stop=False)
                        nc.tensor.matmul(hp[:, 0:qw],
                                         w2r_sb[:, 0, s * P:(s + 1) * P],
                                         h1T_res[:, 0, q0:q0 + qw],
                                         start=False, stop=False)
                        nc.tensor.matmul(hp[:, 0:qw],
                                         w2r_sb[:, 1, s * P:(s + 1) * P],
                                         h1T_res[:, 1, q0:q0 + qw],
                                         start=False, stop=True)
                        if s % 2 == 0:
                            nc.scalar.activation(hT[:, s, 0:qw], hp[:, 0:qw],
                                                 RELU,
                                                 bias=b2t_sb[:, s:s + 1])
                        else:
                            nc.vector.tensor_scalar(hT[:, s, 0:qw],
                                                    hp[:, 0:qw],
                                                    b2t_sb[:, s:s + 1],
                                                    zero_sb[:, 0:1], ADD, MAX)
                    # t3/r3 for the blocks inside this column group
                    for bq in range(qw // P):
                        b = q0 // P + bq
                        emit_t3(b, hT, bq * P)

            # ====== Layer 3 partial aggregation (src-sharded) ======
            if "l3a" not in abl:
                binsts = [[] for _ in range(NBLK)]
                for k, (c, g, _, _) in enumerate(INSTS):
                    binsts[g].append((k, c))
                last_win = [max(c for _, c in bi) // 8 for bi in binsts]
                wtiles = {}
                band_t = None
                w_done = 0
                NW = (NCH + 7) // 8
                for g in range(NBLK):
                    while w_done <= last_win[g] and w_done < NW:
                        c0 = w_done * 8
                        sz = min(8, NCH - c0)
                        gt = gp.tile([P, 8, P // 2], f32, name="gath3",
                                     tag="gath")
                        nc.gpsimd.dma_gather(
                            gt[:, 0:sz, :], t3_own[:].bitcast(f32),
                            gidx2_sb[:, c0 * 8:(c0 + sz) * 8],
                            sz * P, sz * P, P // 2, single_packet=False)
                        wtiles[w_done] = gt.bitcast(f8)
                        w_done += 1
                    ps = psA.tile([P, 2, P], f32, name="agg", tag="agg")
                    n = len(binsts[g])
                    for j, (k, c) in enumerate(binsts[g]):
                        nc.tensor.matmul(ps[:, 0, 0:D_OUT],
                                         ohc8[:, k, :],
                                         wtiles[c // 8][:, c % 8, 0:D_OUT],
                                         start=(j == 0), stop=(j == n - 1))
                    r_band, gl = divmod(g, NB)
                    if gl == 0:
                        band_t = b3p.tile([P, NB, D_OUT], bf16, tag="band3")
                    if gl % 2 == 0:
                        nc.scalar.activation(
                            band_t[:, gl, :], ps[:, 0, 0:D_OUT], COPY)
                    else:
                        nc.vector.tensor_copy(
                            band_t[:, gl, :], ps[:, 0, 0:D_OUT])
                    if gl == NB - 1:
                        nc.sync.dma_start(p3d[r_band], band_t[:])

            if "rs2" not in abl:
                nc.gpsimd.collective_compute(
                    "ReduceScatter", mybir.AluOpType.add,
                    replica_groups=[list(range(NCORES))],
                    ins=[p3d.opt()], outs=[m3d.opt()])

            # ====== final: y = mean3 + r3, log_softmax, store ======
            if "fin" not in abl:
                m3_sb = cp.tile([P, NB, D_OUT], bf16, tag="m3s")
                nc.sync.dma_start(m3_sb[:], m3d[:])
                m3f = cp.tile([P, NB, D_OUT], f32, tag="m3f")
                nc.vector.tensor_copy(m3f[:], m3_sb[:])
                ys, negs = [], []
                ssum_all = cp.tile([P, NB], f32, tag="ssum_all")
                ls_all = cp.tile([P, NB], f32, tag="ls_all")
                ob_all = cp.tile([P, NB, D_OUT], f32, tag="ob_all")
                for b in range(NB):
                    ym = sfp.tile([P, D_OUT], f32, tag="ym")
                    nc.vector.tensor_scalar_mul(ym[:], m3f[:, b, :],
                                                ivown_sb[:, b:b + 1])
                    y = sfp.tile([P, D_OUT], f32, tag="y")
                    nc.vector.tensor_tensor(y[:], ym[:],
                                            r3_res[:, b, :], ADD)
                    negm = sfp.tile([P, 1], f32, tag="negm")
                    nc.vector.tensor_reduce(negm[:], y[:], AXX, MAX,
                                            negate=True)
                    ys.append(y)
                    negs.append(negm)
                for b in range(NB):        # all Exp together (one act table)
                    e = sfp.tile([P, D_OUT], f32, tag="e")
                    nc.scalar.activation(e[:], ys[b][:], EXP,
                                         bias=negs[b][:, 0:1],
                                         scale=1.0,
                                         accum_out=ssum_all[:, b:b + 1])
                nc.scalar.activation(ls_all[:], ssum_all[:], LN)
                for b in range(NB):
                    nc.vector.tensor_scalar(ob_all[:, b, :], ys[b][:],
                                            negs[b][:, 0:1],
                                            ls_all[:, b:b + 1],
                                            ADD, SUB)
                    # partition-major output, spread over 3 DMA queues
                    if b == 3:
                        nc.sync.dma_start(outp[:, 0:4 * D_OUT],
                                          ob_all[:, 0:4, :])
                    elif b == 6:
                        nc.scalar.dma_start(outp[:, 4 * D_OUT:7 * D_OUT],
                                            ob_all[:, 4:7, :])
                    elif b == NB - 1:
                        nc.gpsimd.dma_start(outp[:, 7 * D_OUT:],
                                            ob_all[:, 7:, :])

    nc.compile()
    return nc


def _wrap16(a):
    """idx i -> partition i%16, col i//16; replicated to 128 partitions."""
    w = a.reshape(-1, 16).T
    return np.ascontiguousarray(np.tile(w, (8, 1)))


def _balanced_perm(deg):
    """Assign nodes to 80 blocks of 128 so block in-degree sums are even."""
    import heapq
    order = np.argsort(-deg, kind="stable")
    heap = [(0, 0, g) for g in range(NBLK)]
    heapq.heapify(heap)
    newpos = np.empty(NPAD, np.int64)
    fill = np.zeros(NBLK, np.int64)
    for n in order:
        s, _, g = heapq.heappop(heap)
        newpos[n] = g * P + fill[g]
        fill[g] += 1
        if fill[g] < P:
            heapq.heappush(heap, (s + int(deg[n]), int(fill[g]), g))
    return newpos


def _prep(x, edge_index):
    src = np.asarray(edge_index[0], dtype=np.int64)
    dst = np.asarray(edge_index[1], dtype=np.int64)
    deg = np.bincount(dst, minlength=NPAD).astype(np.float64)
    invdeg_n = (1.0 / np.maximum(deg, 1.0)).astype(np.float32)

    newpos = _balanced_perm(deg)
    oldnode = np.empty(NPAD, np.int64)
    oldnode[newpos] = np.arange(NPAD)
    psrc = newpos[src]
    pdst = newpos[dst]

    # ---------- L1: dst-sharded per-block chunks (global src gather) ----
    order = np.argsort(pdst, kind="stable")
    dsts = pdst[order]
    srcs = psrc[order]
    inv_e = invdeg_n[dst[order]]
    starts = np.searchsorted(dsts, np.arange(0, NPAD + P, P))
    cnt = starts[1:] - starts[:-1]
    MC = max(1, int(np.ceil(cnt.max() / P)))

    l1_per_core = []
    for r in range(NCORES):
        gparts, dparts, iparts = [], [], []
        for j in range(NB):
            g = r * NB + j
            lo, hi = starts[g], starts[g + 1]
            n = hi - lo
            o2 = lo + np.argsort(srcs[lo:hi], kind="stable")
            sg = np.zeros(MC * P, dtype=np.int16)
            dg = np.full(MC * P, -1.0, dtype=np.float32)
            ig = np.zeros(MC * P, dtype=np.float32)
            sg[:n] = srcs[o2].astype(np.int16)
            dg[:n] = (dsts[o2] - g * P).astype(np.float32)
            ig[:n] = inv_e[o2]
            gparts.append(_wrap16(sg))
            dparts.append(np.ascontiguousarray(dg.reshape(MC, P).T))
            iparts.append(np.ascontiguousarray(ig.reshape(MC, P).T))
        l1_per_core.append((
            np.concatenate(gparts, axis=1),
            np.concatenate(dparts, axis=1),
            np.concatenate(iparts, axis=1),
        ))

    # ---------- L2/L3: src-sharded shared schedule (local src gather) ---
    src_core = psrc // PER_CORE
    srcloc = psrc % PER_CORE
    g_all = pdst // P
    dloc_all = (pdst % P).astype(np.float32)
    inv_all = invdeg_n[dst]

    E = np.zeros((NCORES, NBLK), dtype=np.int64)
    for r in range(NCORES):
        E[r] = np.bincount(g_all[src_core == r], minlength=NBLK)
    S = np.maximum(1, E.max(axis=0))          # slots per block (static)
    offs = np.concatenate([[0], np.cumsum(S)])
    T = int(offs[-1])
    NCH = (T + P - 1) // P
    Tpad = NCH * P

    NCHsched, INSTS, _ = _mk_schedule(tuple(int(v) for v in S))
    assert NCHsched == NCH
    NINST = len(INSTS)

    l2_per_core = []
    for r in range(NCORES):
        m = src_core == r
        g_r = g_all[m]
        sl_r = srcloc[m]
        dl_r = dloc_all[m]
        iv_r = inv_all[m]
        o2 = np.lexsort((sl_r, g_r))
        g_s, sl_s, dl_s, iv_s = g_r[o2], sl_r[o2], dl_r[o2], iv_r[o2]
        # position within block group
        gstart = np.searchsorted(g_s, np.arange(NBLK))
        within = np.arange(len(g_s)) - gstart[g_s]
        flat_pos = offs[g_s] + within

        sgf = np.zeros(Tpad, dtype=np.int16)
        dlf = np.full(Tpad, -1.0, dtype=np.float32)
        ivf = np.zeros(Tpad, dtype=np.float32)
        sgf[flat_pos] = sl_s.astype(np.int16)
        dlf[flat_pos] = dl_s
        ivf[flat_pos] = iv_s

        # per-instance one-hots [slot, dst-local], invdeg folded in
        dl_i = np.full((NINST, P), -1.0, dtype=np.float32)
        iv_i = np.zeros((NINST, P), dtype=np.float32)
        for k, (c, g, _, _) in enumerate(INSTS):
            s0 = c * P
            sel = np.arange(s0, s0 + P)
            inb = (sel >= offs[g]) & (sel < offs[g + 1])
            dl_i[k, inb] = dlf[sel[inb]]
            iv_i[k, inb] = ivf[sel[inb]]
        import ml_dtypes
        ohm = (dl_i[:, :, None]
               == np.arange(P, dtype=np.float32)[None, None, :])
        # 0/1 one-hots [NINST, slot, dst] -> SBUF [slot(part), inst, dst]
        oh8 = np.ascontiguousarray(
            ohm.astype(ml_dtypes.float8_e4m3)
            .transpose(1, 0, 2).reshape(P, NINST * P))

        l2_per_core.append((_wrap16(sgf), oh8))

    xp = np.zeros((NPAD, D_IN), dtype=np.float32)
    xp[:N_NODES] = x
    xp = xp[oldnode]           # permuted node order
    ivp = invdeg_n[oldnode]    # invdeg in permuted node order
    return (xp, ivp, l1_per_core, l2_per_core, MC,
            tuple(int(v) for v in S), newpos)


def _make_in_maps(x, edge_index, w1l, w1r, b1, w2l, w2r, b2, w3l, w3r, b3):
    x = np.ascontiguousarray(np.asarray(x, dtype=np.float32))
    xp, ivp, l1pc, l2pc, MC, S, newpos = _prep(x, np.asarray(edge_index))

    iota = np.tile(np.arange(P, dtype=BF), (P, 1))
    b1v = np.asarray(b1, np.float32).reshape(-1)
    b2v = np.asarray(b2, np.float32).reshape(-1)
    xbf = xp.astype(BF)
    common = {
        "xbf": xbf,
        "w1l": np.asarray(w1l, np.float32).astype(BF),
        "w1r": np.asarray(w1r, np.float32).astype(BF),
        "b1": b1v.reshape(1, D_H1).astype(BF),
        "b1t": np.ascontiguousarray(b1v.reshape(2, P).T),
        "w2l": np.asarray(w2l, np.float32).astype(BF),
        "w2r": np.asarray(w2r, np.float32).astype(BF),
        "b2t": np.ascontiguousarray(b2v.reshape(8, P).T),
        "w3lr": np.ascontiguousarray(np.concatenate(
            [np.asarray(w3l, np.float32), np.asarray(w3r, np.float32)],
            axis=1)).astype(BF),
        "b3pad": np.concatenate(
            [np.zeros(D_OUT, np.float32),
             np.asarray(b3, np.float32).reshape(-1)]).reshape(1, P).astype(BF),
        "iota_in": np.ascontiguousarray(iota),
    }
    in_maps = []
    for r in range(NCORES):
        g1, d1, i1 = l1pc[r]
        g2, o8 = l2pc[r]
        mdict = dict(common)
        mdict["xownT"] = np.ascontiguousarray(
            xbf[r * PER_CORE:(r + 1) * PER_CORE].T)
        mdict["gidx1"] = g1
        mdict["dl1"] = d1
        mdict["iv1"] = i1
        mdict["gidx2"] = g2
        mdict["oh8"] = o8
        ivr = ivp[r * PER_CORE:(r + 1) * PER_CORE].reshape(NB, P)
        mdict["ivown"] = np.ascontiguousarray(ivr.T)
        dvd = np.zeros((NB, P, P), dtype=BF)
        for b in range(NB):
            np.fill_diagonal(dvd[b], ivr[b].astype(BF))
        mdict["dvdiag"] = np.ascontiguousarray(
            dvd.transpose(1, 0, 2).reshape(P, NB * P))
        in_maps.append(mdict)
    return in_maps, (MC, S), newpos


def kernel(x, edge_index, w1l, w1r, b1, w2l, w2r, b2, w3l, w3r, b3):
    global LAST_RESULTS
    import os
    from concourse.bass_utils import run_bass_kernel_spmd

    if os.environ.get("BASS_TRACE"):
        try:
            import antenv.axon_hooks  # noqa: F401
        except ImportError:
            os.environ.pop("BASS_TRACE", None)  # no NTFF hook here

    in_maps, key, newpos = _make_in_maps(x, edge_index, w1l, w1r, b1, w2l,
                                         w2r, b2, w3l, w3r, b3)
    if key not in _CACHE:
        _CACHE[key] = _build(key[0], key[1])
    nc = _CACHE[key]

    res = run_bass_kernel_spmd(nc, in_maps, core_ids=list(range(NCORES)))
    LAST_RESULTS = res
    out = np.concatenate(
        [res.results[r]["out"].reshape(P, NB, D_OUT).transpose(1, 0, 2)
         .reshape(PER_CORE, D_OUT) for r in range(NCORES)], axis=0)
    return np.ascontiguousarray(out[newpos[:N_NODES]])
